# revision 23
# baseline (speedup 1.0000x reference)
"""BiaffineCRF loss kernel for Trainium2 (8 NeuronCores, batch-parallel).

Per-core program (4 batch elements each, tokens ordered (l, b) l-major):
  1. x -> PE-transpose -> xT [h=65(+ones), (c,tok)] c-major
  2. e1/e2 = W' @ xT (bias via augmented ones row), PE; chunk-transposed to
     dense e1d/e2d [(tok16, c8), k] tiles
  3. per-token T = e1_tok^T e2_tok (PE, K=8), exp(T - C) -> expT bf16
  4. CRF forward scan in exp domain: p' = (expT_l^T p) * exp(em_l),
     periodic renorm (sum via PE ones-matvec), log-normalizers accumulated
  5. numerator via indirect-DMA gathers of W1/W2 rows + em elements, DVE
     dot-products, PE ones-reduction
  6. loss = numerator - denominator -> DRAM [4]
"""

import math
import numpy as np
from contextlib import ExitStack

import concourse.bass as bass
import concourse.bacc as bacc
import concourse.tile as tile
from concourse import mybir
from concourse.bass_utils import run_bass_kernel_spmd

AF = mybir.ActivationFunctionType
ALU = mybir.AluOpType
FP32 = mybir.dt.float32
BF16 = mybir.dt.bfloat16
I32 = mybir.dt.int32

B, L, EMBED, K, C, H = 32, 256, 512, 64, 8, 64
NCORES = 8
BPC = B // NCORES          # 4 batch elems per core
NTOK = BPC * L             # 1024 tokens per core, tok = 4*l + b
NCHUNK = NTOK // 128       # 8 token chunks of 128
CSHIFT = 14.6              # constant shift inside exp(T - CSHIFT)
GNORM = 8                  # renormalize scan state every GNORM steps


def build_program(nc: bass.Bass):
    dt = nc.dram_tensor
    x = dt("x", [BPC, L, EMBED], FP32, kind="ExternalInput").ap()
    em = dt("em", [BPC, L, K], FP32, kind="ExternalInput").ap()
    W1 = dt("W1", [K, H], FP32, kind="ExternalInput").ap()
    W2 = dt("W2", [K, H], FP32, kind="ExternalInput").ap()
    b1 = dt("b1", [K], FP32, kind="ExternalInput").ap()
    b2 = dt("b2", [K], FP32, kind="ExternalInput").ap()
    st = dt("st", [K], FP32, kind="ExternalInput").ap()
    en = dt("en", [K], FP32, kind="ExternalInput").ap()
    id128 = dt("id128", [128, 128], FP32, kind="ExternalInput").ap()
    selc = dt("selc", [128, 4 * NCHUNK], FP32, kind="ExternalInput").ap()
    mask128 = dt("mask128", [128, 2], FP32, kind="ExternalInput").ap()
    maskT2 = dt("maskT2", [2, 128], FP32, kind="ExternalInput").ap()
    maskq = dt("maskq", [128, 4], FP32, kind="ExternalInput").ap()
    mf = dt("mf", [128, NCHUNK], FP32, kind="ExternalInput").ap()
    w1gidx = dt("w1gidx", [128, NCHUNK], I32, kind="ExternalInput").ap()
    w2gidx = dt("w2gidx", [128, NCHUNK], I32, kind="ExternalInput").ap()
    emselidx = dt("emselidx", [128, NCHUNK], I32, kind="ExternalInput").ap()
    stidx = dt("stidx", [4, 1], I32, kind="ExternalInput").ap()
    enidx = dt("enidx", [4, 1], I32, kind="ExternalInput").ap()
    em0idx = dt("em0idx", [4, 1], I32, kind="ExternalInput").ap()
    loss = dt("loss", [4], FP32, kind="ExternalOutput").ap()

    with tile.TileContext(nc) as tc, ExitStack() as ctx:
        singles = ctx.enter_context(tc.tile_pool(name="singles", bufs=1))
        xpool = ctx.enter_context(tc.tile_pool(name="xpool", bufs=NCHUNK))
        work = ctx.enter_context(tc.tile_pool(name="work", bufs=3))
        etile = ctx.enter_context(tc.tile_pool(name="etile", bufs=3))
        scanp = ctx.enter_context(tc.tile_pool(name="scanp", bufs=3))
        tiny = ctx.enter_context(tc.tile_pool(name="tiny", bufs=4))
        ps_t = ctx.enter_context(tc.tile_pool(name="ps_t", bufs=2, space="PSUM"))
        ps_a = ctx.enter_context(tc.tile_pool(name="ps_a", bufs=2, space="PSUM"))
        ps_T = ctx.enter_context(tc.tile_pool(name="ps_T", bufs=2, space="PSUM"))
        ps_s = ctx.enter_context(tc.tile_pool(name="ps_s", bufs=2, space="PSUM"))

        # ---- persistent SBUF tensors ----
        zconst = singles.tile([128, 1], FP32)
        nc.vector.memset(zconst, 0.0)
        nc.const_aps.aps[(FP32, 0.0)] = zconst[:]
        negC = singles.tile([128, 1], FP32)
        nc.vector.memset(negC, -CSHIFT)
        ident = singles.tile([128, 128], FP32)
        nc.sync.dma_start(out=ident, in_=id128)
        selc_sb = singles.tile([128, 4 * NCHUNK], FP32)
        nc.sync.dma_start(out=selc_sb, in_=selc)
        mask128f = singles.tile([128, 2], FP32)
        nc.sync.dma_start(out=mask128f, in_=mask128)
        mask128b = singles.tile([128, 2], BF16)
        nc.vector.tensor_copy(mask128b, mask128f)
        maskT2_sb = singles.tile([2, 128], FP32)
        nc.sync.dma_start(out=maskT2_sb, in_=maskT2)
        maskq_sb = singles.tile([128, 4], FP32)
        nc.sync.dma_start(out=maskq_sb, in_=maskq)
        mf_sb = singles.tile([128, NCHUNK], FP32)
        nc.sync.dma_start(out=mf_sb, in_=mf)
        w1gi = singles.tile([128, NCHUNK], I32)
        nc.sync.dma_start(out=w1gi, in_=w1gidx)
        w2gi = singles.tile([128, NCHUNK], I32)
        nc.sync.dma_start(out=w2gi, in_=w2gidx)
        emsi = singles.tile([128, NCHUNK], I32)
        nc.sync.dma_start(out=emsi, in_=emselidx)
        sti = singles.tile([4, 1], I32)
        nc.sync.dma_start(out=sti, in_=stidx)
        eni = singles.tile([4, 1], I32)
        nc.sync.dma_start(out=eni, in_=enidx)
        em0i = singles.tile([4, 1], I32)
        nc.sync.dma_start(out=em0i, in_=em0idx)
        stg = singles.tile([4, 1], FP32)
        nc.gpsimd.indirect_dma_start(
            out=stg, out_offset=None, in_=st[:, None],
            in_offset=bass.IndirectOffsetOnAxis(ap=sti, axis=0),
        )
        eng = singles.tile([4, 1], FP32)
        nc.gpsimd.indirect_dma_start(
            out=eng, out_offset=None, in_=en[:, None],
            in_offset=bass.IndirectOffsetOnAxis(ap=eni, axis=0),
        )
        em_flat0 = em.rearrange("b l k -> (b l k)")[:, None]
        em0g = singles.tile([4, 1], FP32)
        nc.gpsimd.indirect_dma_start(
            out=em0g, out_offset=None, in_=em_flat0,
            in_offset=bass.IndirectOffsetOnAxis(ap=em0i, axis=0),
        )

        # W1T' = [W1^T ; b1] : [65, 64]
        W1T = singles.tile([H + 1, K], FP32)
        nc.sync.dma_start(out=W1T[:H, :], in_=W1.rearrange("k h -> h k"))
        nc.sync.dma_start(out=W1T[H : H + 1, :], in_=b1[None, :])
        W2T = singles.tile([H + 1, K], FP32)
        nc.sync.dma_start(out=W2T[:H, :], in_=W2.rearrange("k h -> h k"))
        nc.sync.dma_start(out=W2T[H : H + 1, :], in_=b2[None, :])

        st_sb = singles.tile([K, 1], FP32)
        nc.sync.dma_start(out=st_sb, in_=st[:, None])
        en2 = singles.tile([128, 1], FP32)
        nc.sync.dma_start(out=en2[0:64, :], in_=en[:, None])
        nc.sync.dma_start(out=en2[64:128, :], in_=en[:, None])
        ee = singles.tile([128, 1], FP32)
        nc.scalar.activation(ee, en2, AF.Exp)
        expEnd2 = singles.tile([128, 2], BF16)
        nc.vector.tensor_tensor(
            out=expEnd2, in0=ee.to_broadcast([128, 2]), in1=mask128f, op=ALU.mult
        )

        # ---- load x chunks (tok-major (l,b)) and em chunks ----
        # chunk i holds tokens i*128 .. i*128+127, i.e. l in [i*32, i*32+32), all 4 b
        x_sb = []
        xf = x.rearrange("b l d -> (b l) d")
        for i in range(NCHUNK):
            xt = xpool.tile([128, EMBED], FP32, tag="xchunk")
            nc.sync.dma_start(out=xt, in_=xf[i * 128 : (i + 1) * 128, :])
            x_sb.append(xt)

        # xT' [65, C*NTOK] grouped: col = (tok//64)*512 + c*64 + tok%64
        xT = singles.tile([H + 1, C * NTOK], FP32)
        nc.vector.memset(xT[H : H + 1, :], 1.0)
        xTg = xT.rearrange("h (g t c) -> h g t c", t=64, c=C)
        for i in range(NCHUNK):
            for c in range(C):
                pt = ps_t.tile([H, 128], FP32, tag="tp")
                nc.tensor.transpose(
                    out=pt, in_=x_sb[i][:, c * H : (c + 1) * H], identity=ident
                )
                nc.scalar.activation(
                    xTg[:H, 2 * i : 2 * i + 2, :, c], pt, AF.Copy
                )

        # em transposed: emT [64 j, NTOK] ; expEm = exp(emT)
        emT = singles.tile([K, NTOK], FP32)
        for i in range(NCHUNK):
            emt = work.tile([128, K], FP32, tag="emchunk")
            emf = em.rearrange("b l k -> (b l) k")
            nc.sync.dma_start(out=emt, in_=emf[i * 128 : (i + 1) * 128, :])
            pt = ps_t.tile([K, 128], FP32, tag="tp")
            nc.tensor.transpose(out=pt, in_=emt, identity=ident)
            nc.scalar.activation(emT[:, i * 128 : (i + 1) * 128], pt, AF.Copy)
        expEmD = singles.tile([128, NTOK], FP32)
        for i in range(2):
            nc.scalar.activation(
                expEmD[0:64, i * 512 : (i + 1) * 512],
                emT[:, i * 512 : (i + 1) * 512], AF.Exp,
            )
        nc.sync.dma_start(out=expEmD[64:128, :], in_=expEmD[0:64, :])

        # ---- e1/e2 -> dense [(tok,c), k] tiles; then per-token T and expT ----
        # e1d/e2d: 64 tiles of [128=(16 tok, 8 c), 64 k] stored as [128, 64*64]
        e1d = singles.tile([128, NTOK // 16 * K], BF16)
        e2z = [
            singles.tile([128, NTOK // 16 * K], BF16, name=f"e2z{i}", tag=f"e2z{i}")
            for i in range(4)
        ]
        for g in range(NTOK // 64):  # 16 groups of 64 tokens
            rhs = xT[:, g * 512 : (g + 1) * 512]  # [65, 512] cols (t64, c)
            for w, wt in ((0, W1T), (1, W2T)):
                pe = ps_a.tile([K, 512], FP32, tag="pa")
                nc.tensor.matmul(out=pe, lhsT=wt, rhs=rhs, start=True, stop=True)
                se = etile.tile([K, 512], FP32, tag=f"se{w}")
                nc.scalar.activation(se, pe, AF.Copy)
                # transpose quarters: [64k, 128=(16t,8c)] -> [128, 64]
                for q in range(4):
                    pt = ps_t.tile([128, K], FP32, tag="tp")
                    nc.tensor.transpose(
                        out=pt, in_=se[:, q * 128 : (q + 1) * 128],
                        identity=ident[:K, :K],
                    )
                    j = g * 4 + q
                    if w == 0:
                        nc.vector.tensor_copy(e1d[:, j * K : (j + 1) * K], pt)
                    else:
                        for qq in range(4):
                            nc.vector.tensor_tensor(
                                out=e2z[qq][:, j * K : (j + 1) * K], in0=pt,
                                in1=maskq_sb[:, qq : qq + 1].to_broadcast([128, K]),
                                op=ALU.mult,
                            )

        # expT [128, L*128] bf16: step l owns cols l*128..+128; token (l,b) at
        # half h, col-block pos:  l odd: (h,pos)=(b%2, b//2)  l even: (b//2, b%2)
        expT = singles.tile([128, L * 128], BF16)
        for bank in range(L // 4):  # 64 banks of 4 steps = 16 tokens
            pT = ps_T.tile([128, 512], FP32, tag="pT")
            for li in range(4):
                l = bank * 4 + li
                for b in range(BPC):
                    h, pos = (b % 2, b // 2) if l % 2 else (b // 2, b % 2)
                    ci = b * L + l
                    j = ci // 16
                    r32 = ((ci % 16) // 4) * 32
                    col = li * 128 + pos * 64
                    nc.tensor.matmul(
                        out=pT[h * 64 : h * 64 + 64, col : col + 64],
                        lhsT=e1d[r32 : r32 + 32, j * K : (j + 1) * K],
                        rhs=e2z[ci % 4][r32 : r32 + 32, j * K : (j + 1) * K],
                        start=True,
                        stop=True,
                        tile_position=(r32, h * 64),
                    )
            nc.scalar.activation(
                expT[:, bank * 512 : (bank + 1) * 512], pT, AF.Exp, bias=negC[:]
            )

        # ---- scan ----
        # state pL [128, 2]: half0 = (pos0, pos1) tokens of lhsT half 0, etc.
        logacc = singles.tile([2, 2], FP32)
        nc.vector.memset(logacc, (L - 1) * CSHIFT)
        p0 = tiny.tile([K, 4], BF16, tag="p0")
        emT4 = emT.rearrange("j (b t) -> j b t", b=BPC)
        nc.scalar.activation(p0, emT4[:, :, 0], AF.Exp, bias=st_sb)
        p03 = p0.rearrange("i (a b) -> i b a", b=2)
        pL = scanp.tile([128, 2], BF16, tag="p")
        nc.sync.dma_start(out=pL[0:64, :], in_=p03[:, 0, :])
        nc.sync.dma_start(out=pL[64:128, :], in_=p03[:, 1, :])

        eD4 = expEmD.rearrange("j (b t) -> j b t", b=BPC)
        eD8 = expEmD.rearrange("j (g two t) -> j two g t", g=2, two=2)
        for l in range(1, L):
            ps4 = ps_s.tile([128, 4], FP32, tag="ps")
            nc.tensor.matmul(
                out=ps4[:, 0:2], lhsT=expT[0:64, l * 128 : l * 128 + 128],
                rhs=pL[0:64, :], start=True, stop=True,
            )
            nc.tensor.matmul(
                out=ps4[:, 2:4], lhsT=expT[64:128, l * 128 : l * 128 + 128],
                rhs=pL[64:128, :], start=True, stop=True,
            )
            pn = scanp.tile([128, 2], BF16, tag="p")
            ps4v = ps4.rearrange("p (a b) -> p b a", b=2)
            if l % 2 == 1:
                em1 = eD4[0:64, 0:2, l]      # (b0, b1)
                em2 = eD4[64:128, 2:4, l]    # (b2, b3)
            else:
                em1 = eD8[0:64, 0, :, l]     # (b0, b2)
                em2 = eD8[64:128, 1, :, l]   # (b1, b3)
            nc.vector.tensor_tensor(
                out=pn[0:64, :], in0=ps4v[0:64, 0, :], in1=em1, op=ALU.mult
            )
            nc.vector.tensor_tensor(
                out=pn[64:128, :], in0=ps4v[64:128, 1, :], in1=em2, op=ALU.mult
            )
            pL = pn

            if l % GNORM == 0:
                psS = ps_s.tile([2, 2], FP32, tag="ps")
                nc.tensor.matmul(out=psS, lhsT=mask128b, rhs=pL, start=True, stop=True)
                logS = tiny.tile([2, 2], FP32, tag="logS")
                nc.scalar.activation(logS, psS, AF.Ln)
                nc.vector.tensor_tensor(out=logacc, in0=logacc, in1=logS, op=ALU.add)
                r22 = tiny.tile([2, 2], FP32, tag="r22")
                nc.vector.reciprocal(r22, psS)
                psR = ps_s.tile([128, 2], FP32, tag="ps")
                nc.tensor.matmul(out=psR, lhsT=maskT2_sb, rhs=r22, start=True, stop=True)
                rb = tiny.tile([128, 2], BF16, tag="rb")
                nc.vector.tensor_copy(rb, psR)
                pn2 = scanp.tile([128, 2], BF16, tag="p")
                nc.vector.tensor_tensor(out=pn2, in0=pL, in1=rb, op=ALU.mult)
                pL = pn2

        psE = ps_s.tile([2, 2], FP32, tag="ps")
        nc.tensor.matmul(out=psE, lhsT=pL, rhs=expEnd2, start=True, stop=True)
        logU = tiny.tile([2, 2], FP32, tag="logU")
        nc.scalar.activation(logU, psE, AF.Ln)
        den22 = tiny.tile([2, 2], FP32, tag="den22")
        nc.vector.tensor_tensor(out=den22, in0=logacc, in1=logU, op=ALU.add)
        den4 = tiny.tile([4, 1], FP32, tag="den4")
        nc.sync.dma_start(out=den4[0:2, :], in_=den22[:, 0:1])
        nc.sync.dma_start(out=den4[2:4, :], in_=den22[:, 1:2])

        # ---- numerator ----
        psN = ps_s.tile([4, 1], FP32, tag="ps")
        em_flat = em.rearrange("b l k -> (b l k)")[:, None]
        for i in range(NCHUNK):
            w1g = work.tile([128, H], FP32, tag="w1g")
            nc.gpsimd.indirect_dma_start(
                out=w1g, out_offset=None, in_=W1,
                in_offset=bass.IndirectOffsetOnAxis(ap=w1gi[:, i : i + 1], axis=0),
            )
            w2g = work.tile([128, H], FP32, tag="w2g")
            nc.gpsimd.indirect_dma_start(
                out=w2g, out_offset=None, in_=W2,
                in_offset=bass.IndirectOffsetOnAxis(ap=w2gi[:, i : i + 1], axis=0),
            )
            emg = work.tile([128, 1], FP32, tag="emg")
            nc.gpsimd.indirect_dma_start(
                out=emg, out_offset=None, in_=em_flat,
                in_offset=bass.IndirectOffsetOnAxis(ap=emsi[:, i : i + 1], axis=0),
            )
            x3 = x_sb[i].rearrange("p (c h) -> p c h", c=C)
            prod = work.tile([128, C, H], FP32, tag="prod")
            nc.vector.tensor_tensor(
                out=prod, in0=x3, in1=w1g[:, None, :].to_broadcast([128, C, H]),
                op=ALU.mult,
            )
            A = work.tile([128, C], FP32, tag="A")
            nc.vector.reduce_sum(out=A, in_=prod, axis=mybir.AxisListType.X)
            prod2 = work.tile([128, C, H], FP32, tag="prod")
            nc.vector.tensor_tensor(
                out=prod2, in0=x3, in1=w2g[:, None, :].to_broadcast([128, C, H]),
                op=ALU.mult,
            )
            Bt = work.tile([128, C], FP32, tag="B")
            nc.vector.reduce_sum(out=Bt, in_=prod2, axis=mybir.AxisListType.X)
            AB = work.tile([128, C], FP32, tag="AB")
            nc.vector.tensor_tensor(out=AB, in0=A, in1=Bt, op=ALU.mult)
            tsel = work.tile([128, 1], FP32, tag="tsel")
            nc.vector.reduce_sum(out=tsel, in_=AB, axis=mybir.AxisListType.X)
            ntok = work.tile([128, 1], FP32, tag="ntok")
            nc.vector.tensor_tensor(out=ntok, in0=tsel, in1=emg, op=ALU.add)
            nc.vector.tensor_tensor(
                out=ntok, in0=ntok, in1=mf_sb[:, i : i + 1], op=ALU.mult
            )
            nc.tensor.matmul(
                out=psN, lhsT=selc_sb[:, i * 4 : (i + 1) * 4], rhs=ntok,
                start=(i == 0), stop=(i == NCHUNK - 1),
            )

        p1 = tiny.tile([4, 1], FP32, tag="p1")
        nc.vector.tensor_tensor(out=p1, in0=stg, in1=eng, op=ALU.add)
        nc.vector.tensor_tensor(out=p1, in0=p1, in1=em0g, op=ALU.add)
        numt = tiny.tile([4, 1], FP32, tag="numt")
        nc.vector.tensor_tensor(out=numt, in0=psN, in1=p1, op=ALU.add)
        res = tiny.tile([4, 1], FP32, tag="res")
        nc.vector.tensor_tensor(out=res, in0=numt, in1=den4, op=ALU.subtract)
        nc.sync.dma_start(out=loss[:, None], in_=res)

    return nc


def make_core_inputs(inputs, emissions, targets, masks, W1, b1, W2, b2,
                     start_transitions, end_transitions, core):
    b0 = core * BPC
    x = np.ascontiguousarray(inputs[b0 : b0 + BPC]).astype(np.float32)
    em = np.ascontiguousarray(emissions[b0 : b0 + BPC]).astype(np.float32)
    tg = targets[b0 : b0 + BPC].astype(np.int64)
    mk = masks[b0 : b0 + BPC].astype(bool)

    # token col index: ci = b_local*L + l ; chunk i holds ci in [i*128, (i+1)*128)
    bs = np.repeat(np.arange(BPC), L)          # [NTOK]
    ls = np.tile(np.arange(L), BPC)            # [NTOK]
    tgn = tg[bs, ls].astype(np.int32)          # tg[b, l]
    lprev = np.maximum(ls - 1, 0)
    tgp = tg[bs, lprev].astype(np.int32)       # tg[b, l-1] (l=0 dummy)
    mfv = mk[bs, ls].astype(np.float32)
    mfv[ls == 0] = 0.0
    emsel = ((bs * L + ls) * K + tgn).astype(np.int32)

    def colmaj(v, dtype):
        return np.ascontiguousarray(v.reshape(NCHUNK, 128).T.copy()).astype(dtype)

    seq_ends = mk.astype(np.int64).sum(axis=1) - 1
    id128 = np.eye(128, dtype=np.float32)
    selc = np.zeros((128, NCHUNK, 4), dtype=np.float32)
    for i in range(NCHUNK):
        selc[:, i, i // 2] = 1.0
    selc = selc.reshape(128, NCHUNK * 4)
    return {
        "x": x,
        "em": em,
        "W1": W1.astype(np.float32), "W2": W2.astype(np.float32),
        "b1": b1.astype(np.float32), "b2": b2.astype(np.float32),
        "st": start_transitions.astype(np.float32),
        "en": end_transitions.astype(np.float32),
        "id128": id128,
        "selc": selc,
        "mask128": np.kron(np.eye(2), np.ones((64, 1))).astype(np.float32),
        "maskq": np.kron(np.ones((4, 1)), np.kron(np.eye(4), np.ones((8, 1)))).astype(np.float32),
        "maskT2": np.kron(np.eye(2), np.ones((1, 64))).astype(np.float32),
        "mf": colmaj(mfv, np.float32),
        "w1gidx": colmaj(tgp, np.int32),
        "w2gidx": colmaj(tgn, np.int32),
        "emselidx": colmaj(emsel, np.int32),
        "stidx": tg[:, 0].astype(np.int32).reshape(4, 1),
        "enidx": tg[np.arange(BPC), seq_ends].astype(np.int32).reshape(4, 1),
        "em0idx": ((np.arange(BPC) * L + 0) * K + tg[:, 0]).astype(np.int32).reshape(4, 1),
    }


_trace = False


def kernel(**inputs):
    nc = bacc.Bacc("TRN2", target_bir_lowering=False, num_devices=NCORES)
    build_program(nc)
    nc.finalize()
    args = {k: np.asarray(v) for k, v in inputs.items()}
    in_maps = [
        make_core_inputs(
            args["inputs"], args["emissions"], args["targets"], args["masks"],
            args["W1"], args["b1"], args["W2"], args["b2"],
            args["start_transitions"], args["end_transitions"], core,
        )
        for core in range(NCORES)
    ]
    res = run_bass_kernel_spmd(nc, in_maps, core_ids=list(range(NCORES)),
                               trace=_trace)
    out = np.concatenate([res.results[c]["loss"] for c in range(NCORES)])
    if _trace:
        kernel.last_results = res
    return out.astype(np.float32)


# revision 24
# speedup vs baseline: 2.0420x; 2.0420x over previous
"""BiaffineCRF loss kernel for Trainium2 (8 NeuronCores, batch-parallel).

Per-core program (4 batch elements each, tokens ordered (l, b) l-major):
  1. x -> PE-transpose -> xT [h=65(+ones), (c,tok)] c-major
  2. e1/e2 = W' @ xT (bias via augmented ones row), PE; chunk-transposed to
     dense e1d/e2d [(tok16, c8), k] tiles
  3. per-token T = e1_tok^T e2_tok (PE, K=8), exp(T - C) -> expT bf16
  4. CRF forward scan in exp domain: p' = (expT_l^T p) * exp(em_l),
     periodic renorm (sum via PE ones-matvec), log-normalizers accumulated
  5. numerator via indirect-DMA gathers of W1/W2 rows + em elements, DVE
     dot-products, PE ones-reduction
  6. loss = numerator - denominator -> DRAM [4]
"""

import math
import numpy as np
from contextlib import ExitStack

import concourse.bass as bass
import concourse.bacc as bacc
import concourse.tile as tile
from concourse import mybir
from concourse.bass_utils import run_bass_kernel_spmd

AF = mybir.ActivationFunctionType
ALU = mybir.AluOpType
FP32 = mybir.dt.float32
BF16 = mybir.dt.bfloat16
I32 = mybir.dt.int32

B, L, EMBED, K, C, H = 32, 256, 512, 64, 8, 64
NCORES = 8
BPC = B // NCORES          # 4 batch elems per core
NTOK = BPC * L             # 1024 tokens per core, tok = 4*l + b
NCHUNK = NTOK // 128       # 8 token chunks of 128
CSHIFT = 14.6              # constant shift inside exp(T - CSHIFT)
GNORM = 8                  # renormalize scan state every GNORM steps


def build_program(nc: bass.Bass):
    dt = nc.dram_tensor
    x = dt("x", [BPC, L, EMBED], FP32, kind="ExternalInput").ap()
    em = dt("em", [BPC, L, K], FP32, kind="ExternalInput").ap()
    W1 = dt("W1", [K, H], FP32, kind="ExternalInput").ap()
    W2 = dt("W2", [K, H], FP32, kind="ExternalInput").ap()
    b1 = dt("b1", [K], FP32, kind="ExternalInput").ap()
    b2 = dt("b2", [K], FP32, kind="ExternalInput").ap()
    st = dt("st", [K], FP32, kind="ExternalInput").ap()
    en = dt("en", [K], FP32, kind="ExternalInput").ap()
    id128 = dt("id128", [128, 128], FP32, kind="ExternalInput").ap()
    selc = dt("selc", [128, 4 * NCHUNK], FP32, kind="ExternalInput").ap()
    mask128 = dt("mask128", [128, 2], FP32, kind="ExternalInput").ap()
    maskT2 = dt("maskT2", [2, 128], FP32, kind="ExternalInput").ap()
    maskq = dt("maskq", [128, 4], FP32, kind="ExternalInput").ap()
    mf = dt("mf", [128, NCHUNK], FP32, kind="ExternalInput").ap()
    w1gidx = dt("w1gidx", [128, NCHUNK], I32, kind="ExternalInput").ap()
    w2gidx = dt("w2gidx", [128, NCHUNK], I32, kind="ExternalInput").ap()
    emselidx = dt("emselidx", [128, NCHUNK], I32, kind="ExternalInput").ap()
    stidx = dt("stidx", [4, 1], I32, kind="ExternalInput").ap()
    enidx = dt("enidx", [4, 1], I32, kind="ExternalInput").ap()
    em0idx = dt("em0idx", [4, 1], I32, kind="ExternalInput").ap()
    loss = dt("loss", [4], FP32, kind="ExternalOutput").ap()

    with tile.TileContext(nc) as tc, ExitStack() as ctx:
        singles = ctx.enter_context(tc.tile_pool(name="singles", bufs=1))
        xpool = ctx.enter_context(tc.tile_pool(name="xpool", bufs=NCHUNK))
        work = ctx.enter_context(tc.tile_pool(name="work", bufs=3))
        etile = ctx.enter_context(tc.tile_pool(name="etile", bufs=3))
        scanp = ctx.enter_context(tc.tile_pool(name="scanp", bufs=3))
        tiny = ctx.enter_context(tc.tile_pool(name="tiny", bufs=4))
        ps_t = ctx.enter_context(tc.tile_pool(name="ps_t", bufs=2, space="PSUM"))
        ps_a = ctx.enter_context(tc.tile_pool(name="ps_a", bufs=2, space="PSUM"))
        ps_T = ctx.enter_context(tc.tile_pool(name="ps_T", bufs=2, space="PSUM"))
        ps_s = ctx.enter_context(tc.tile_pool(name="ps_s", bufs=2, space="PSUM"))

        # ---- persistent SBUF tensors ----
        zconst = singles.tile([128, 1], FP32)
        nc.vector.memset(zconst, 0.0)
        nc.const_aps.aps[(FP32, 0.0)] = zconst[:]
        negC = singles.tile([128, 1], FP32)
        nc.vector.memset(negC, -CSHIFT)
        ident = singles.tile([128, 128], FP32)
        nc.sync.dma_start(out=ident, in_=id128)
        selc_sb = singles.tile([128, 4 * NCHUNK], FP32)
        nc.sync.dma_start(out=selc_sb, in_=selc)
        mask128f = singles.tile([128, 2], FP32)
        nc.sync.dma_start(out=mask128f, in_=mask128)
        mask128b = singles.tile([128, 2], BF16)
        nc.vector.tensor_copy(mask128b, mask128f)
        maskT2_sb = singles.tile([2, 128], FP32)
        nc.sync.dma_start(out=maskT2_sb, in_=maskT2)
        maskq_sb = singles.tile([128, 4], FP32)
        nc.sync.dma_start(out=maskq_sb, in_=maskq)
        mf_sb = singles.tile([128, NCHUNK], FP32)
        nc.sync.dma_start(out=mf_sb, in_=mf)
        w1gi = singles.tile([128, NCHUNK], I32)
        nc.sync.dma_start(out=w1gi, in_=w1gidx)
        w2gi = singles.tile([128, NCHUNK], I32)
        nc.sync.dma_start(out=w2gi, in_=w2gidx)
        emsi = singles.tile([128, NCHUNK], I32)
        nc.sync.dma_start(out=emsi, in_=emselidx)
        sti = singles.tile([4, 1], I32)
        nc.sync.dma_start(out=sti, in_=stidx)
        eni = singles.tile([4, 1], I32)
        nc.sync.dma_start(out=eni, in_=enidx)
        em0i = singles.tile([4, 1], I32)
        nc.sync.dma_start(out=em0i, in_=em0idx)
        stg = singles.tile([4, 1], FP32)
        nc.gpsimd.indirect_dma_start(
            out=stg, out_offset=None, in_=st[:, None],
            in_offset=bass.IndirectOffsetOnAxis(ap=sti, axis=0),
        )
        eng = singles.tile([4, 1], FP32)
        nc.gpsimd.indirect_dma_start(
            out=eng, out_offset=None, in_=en[:, None],
            in_offset=bass.IndirectOffsetOnAxis(ap=eni, axis=0),
        )
        em_flat0 = em.rearrange("b l k -> (b l k)")[:, None]
        em0g = singles.tile([4, 1], FP32)
        nc.gpsimd.indirect_dma_start(
            out=em0g, out_offset=None, in_=em_flat0,
            in_offset=bass.IndirectOffsetOnAxis(ap=em0i, axis=0),
        )

        # W1T' = [W1^T ; b1] : [65, 64]
        W1T = singles.tile([H + 1, K], FP32)
        nc.sync.dma_start(out=W1T[:H, :], in_=W1.rearrange("k h -> h k"))
        nc.sync.dma_start(out=W1T[H : H + 1, :], in_=b1[None, :])
        W2T = singles.tile([H + 1, K], FP32)
        nc.sync.dma_start(out=W2T[:H, :], in_=W2.rearrange("k h -> h k"))
        nc.sync.dma_start(out=W2T[H : H + 1, :], in_=b2[None, :])

        st_sb = singles.tile([K, 1], FP32)
        nc.sync.dma_start(out=st_sb, in_=st[:, None])
        en2 = singles.tile([128, 1], FP32)
        nc.sync.dma_start(out=en2[0:64, :], in_=en[:, None])
        nc.sync.dma_start(out=en2[64:128, :], in_=en[:, None])
        ee = singles.tile([128, 1], FP32)
        nc.scalar.activation(ee, en2, AF.Exp)
        expEnd2 = singles.tile([128, 2], BF16)
        nc.vector.tensor_tensor(
            out=expEnd2, in0=ee.to_broadcast([128, 2]), in1=mask128f, op=ALU.mult
        )

        # ---- load x chunks (tok-major (l,b)) and em chunks ----
        # chunk i holds tokens i*128 .. i*128+127, i.e. l in [i*32, i*32+32), all 4 b
        x_sb = []
        xf = x.rearrange("b l d -> (b l) d")
        for i in range(NCHUNK):
            xt = xpool.tile([128, EMBED], FP32, tag="xchunk")
            nc.sync.dma_start(out=xt, in_=xf[i * 128 : (i + 1) * 128, :])
            x_sb.append(xt)

        # xT' [65, C*NTOK] grouped: col = (tok//64)*512 + c*64 + tok%64
        xT = singles.tile([H + 1, C * NTOK], FP32)
        nc.vector.memset(xT[H : H + 1, :], 1.0)
        xTg = xT.rearrange("h (g t c) -> h g t c", t=64, c=C)
        for i in range(NCHUNK):
            for c in range(C):
                pt = ps_t.tile([H, 128], FP32, tag="tp")
                nc.tensor.transpose(
                    out=pt, in_=x_sb[i][:, c * H : (c + 1) * H], identity=ident
                )
                nc.scalar.activation(
                    xTg[:H, 2 * i : 2 * i + 2, :, c], pt, AF.Copy
                )

        # em transposed: emT [64 j, NTOK] ; expEm = exp(emT)
        emT = singles.tile([K, NTOK], FP32)
        for i in range(NCHUNK):
            emt = work.tile([128, K], FP32, tag="emchunk")
            emf = em.rearrange("b l k -> (b l) k")
            nc.sync.dma_start(out=emt, in_=emf[i * 128 : (i + 1) * 128, :])
            pt = ps_t.tile([K, 128], FP32, tag="tp")
            nc.tensor.transpose(out=pt, in_=emt, identity=ident)
            nc.scalar.activation(emT[:, i * 128 : (i + 1) * 128], pt, AF.Copy)
        expEmD = singles.tile([128, NTOK], FP32)
        for i in range(2):
            nc.scalar.activation(
                expEmD[0:64, i * 512 : (i + 1) * 512],
                emT[:, i * 512 : (i + 1) * 512], AF.Exp,
            )
        nc.sync.dma_start(out=expEmD[64:128, :], in_=expEmD[0:64, :])

        # ---- e1/e2 -> dense [(tok,c), k] tiles; then per-token T and expT ----
        # e1d/e2d: 64 tiles of [128=(16 tok, 8 c), 64 k] stored as [128, 64*64]
        e1d = singles.tile([128, NTOK // 16 * K], BF16)
        e2z = [
            singles.tile([128, NTOK // 16 * K], BF16, name=f"e2z{i}", tag=f"e2z{i}")
            for i in range(4)
        ]
        for g in range(NTOK // 64):  # 16 groups of 64 tokens
            rhs = xT[:, g * 512 : (g + 1) * 512]  # [65, 512] cols (t64, c)
            for w, wt in ((0, W1T), (1, W2T)):
                pe = ps_a.tile([K, 512], FP32, tag="pa")
                nc.tensor.matmul(out=pe, lhsT=wt, rhs=rhs, start=True, stop=True)
                se = etile.tile([K, 512], FP32, tag=f"se{w}")
                nc.scalar.activation(se, pe, AF.Copy)
                # transpose quarters: [64k, 128=(16t,8c)] -> [128, 64]
                for q in range(4):
                    pt = ps_t.tile([128, K], FP32, tag="tp")
                    nc.tensor.transpose(
                        out=pt, in_=se[:, q * 128 : (q + 1) * 128],
                        identity=ident[:K, :K],
                    )
                    j = g * 4 + q
                    if w == 0:
                        nc.vector.tensor_copy(e1d[:, j * K : (j + 1) * K], pt)
                    else:
                        for qq in range(4):
                            nc.vector.tensor_tensor(
                                out=e2z[qq][:, j * K : (j + 1) * K], in0=pt,
                                in1=maskq_sb[:, qq : qq + 1].to_broadcast([128, K]),
                                op=ALU.mult,
                            )

        # expT [128, L*128] bf16: step l owns cols l*128..+128; token (l,b) at
        # half h, col-block pos:  l odd: (h,pos)=(b%2, b//2)  l even: (b//2, b%2)
        expT = singles.tile([128, L * 128], BF16)
        for bank in range(L // 4):  # 64 banks of 4 steps = 16 tokens
            pT = ps_T.tile([128, 512], FP32, tag="pT")
            for li in range(4):
                l = bank * 4 + li
                for b in range(BPC):
                    h, pos = (b % 2, b // 2) if l % 2 else (b // 2, b % 2)
                    ci = b * L + l
                    j = ci // 16
                    r32 = ((ci % 16) // 4) * 32
                    col = li * 128 + pos * 64
                    nc.tensor.matmul(
                        out=pT[h * 64 : h * 64 + 64, col : col + 64],
                        lhsT=e1d[r32 : r32 + 32, j * K : (j + 1) * K],
                        rhs=e2z[ci % 4][r32 : r32 + 32, j * K : (j + 1) * K],
                        start=True,
                        stop=True,
                        tile_position=(r32, h * 64),
                    )
            nc.scalar.activation(
                expT[:, bank * 512 : (bank + 1) * 512], pT, AF.Exp, bias=negC[:]
            )

        # ---- scan ----
        # state pL [128, 2]: half0 = (pos0, pos1) tokens of lhsT half 0, etc.
        logacc = singles.tile([2, 2], FP32)
        nc.vector.memset(logacc, (L - 1) * CSHIFT)
        p0 = tiny.tile([K, 4], BF16, tag="p0")
        emT4 = emT.rearrange("j (b t) -> j b t", b=BPC)
        nc.scalar.activation(p0, emT4[:, :, 0], AF.Exp, bias=st_sb)
        p03 = p0.rearrange("i (a b) -> i b a", b=2)
        pL = scanp.tile([128, 2], BF16, tag="p")
        nc.sync.dma_start(out=pL[0:64, :], in_=p03[:, 0, :])
        nc.sync.dma_start(out=pL[64:128, :], in_=p03[:, 1, :])

        eD4 = expEmD.rearrange("j (b t) -> j b t", b=BPC)
        eD8 = expEmD.rearrange("j (g two t) -> j two g t", g=2, two=2)
        for l in range(1, L):
            ps4 = ps_s.tile([128, 4], FP32, tag="ps")
            nc.tensor.matmul(
                out=ps4[:, 0:2], lhsT=expT[0:64, l * 128 : l * 128 + 128],
                rhs=pL[0:64, :], start=True, stop=True,
            )
            nc.tensor.matmul(
                out=ps4[:, 2:4], lhsT=expT[64:128, l * 128 : l * 128 + 128],
                rhs=pL[64:128, :], start=True, stop=True,
            )
            pn = scanp.tile([128, 2], BF16, tag="p")
            ps4v = ps4.rearrange("p (a b) -> p b a", b=2)
            if l % 2 == 1:
                em1 = eD4[0:64, 0:2, l]      # (b0, b1)
                em2 = eD4[64:128, 2:4, l]    # (b2, b3)
            else:
                em1 = eD8[0:64, 0, :, l]     # (b0, b2)
                em2 = eD8[64:128, 1, :, l]   # (b1, b3)
            nc.vector.tensor_tensor(
                out=pn[0:64, :], in0=ps4v[0:64, 0, :], in1=em1, op=ALU.mult
            )
            nc.vector.tensor_tensor(
                out=pn[64:128, :], in0=ps4v[64:128, 1, :], in1=em2, op=ALU.mult
            )
            pL = pn

            if l % GNORM == 0:
                psS = ps_s.tile([2, 2], FP32, tag="ps")
                nc.tensor.matmul(out=psS, lhsT=mask128b, rhs=pL, start=True, stop=True)
                logS = tiny.tile([2, 2], FP32, tag="logS")
                nc.scalar.activation(logS, psS, AF.Ln)
                nc.vector.tensor_tensor(out=logacc, in0=logacc, in1=logS, op=ALU.add)
                r22 = tiny.tile([2, 2], FP32, tag="r22")
                nc.vector.reciprocal(r22, psS)
                psR = ps_s.tile([128, 2], FP32, tag="ps")
                nc.tensor.matmul(out=psR, lhsT=maskT2_sb, rhs=r22, start=True, stop=True)
                rb = tiny.tile([128, 2], BF16, tag="rb")
                nc.vector.tensor_copy(rb, psR)
                pn2 = scanp.tile([128, 2], BF16, tag="p")
                nc.vector.tensor_tensor(out=pn2, in0=pL, in1=rb, op=ALU.mult)
                pL = pn2

        psE = ps_s.tile([2, 2], FP32, tag="ps")
        nc.tensor.matmul(out=psE, lhsT=pL, rhs=expEnd2, start=True, stop=True)
        logU = tiny.tile([2, 2], FP32, tag="logU")
        nc.scalar.activation(logU, psE, AF.Ln)
        den22 = tiny.tile([2, 2], FP32, tag="den22")
        nc.vector.tensor_tensor(out=den22, in0=logacc, in1=logU, op=ALU.add)
        den4 = tiny.tile([4, 1], FP32, tag="den4")
        nc.sync.dma_start(out=den4[0:2, :], in_=den22[:, 0:1])
        nc.sync.dma_start(out=den4[2:4, :], in_=den22[:, 1:2])

        # ---- numerator ----
        psN = ps_s.tile([4, 1], FP32, tag="ps")
        em_flat = em.rearrange("b l k -> (b l k)")[:, None]
        for i in range(NCHUNK):
            w1g = work.tile([128, H], FP32, tag="w1g")
            nc.gpsimd.indirect_dma_start(
                out=w1g, out_offset=None, in_=W1,
                in_offset=bass.IndirectOffsetOnAxis(ap=w1gi[:, i : i + 1], axis=0),
            )
            w2g = work.tile([128, H], FP32, tag="w2g")
            nc.gpsimd.indirect_dma_start(
                out=w2g, out_offset=None, in_=W2,
                in_offset=bass.IndirectOffsetOnAxis(ap=w2gi[:, i : i + 1], axis=0),
            )
            emg = work.tile([128, 1], FP32, tag="emg")
            nc.gpsimd.indirect_dma_start(
                out=emg, out_offset=None, in_=em_flat,
                in_offset=bass.IndirectOffsetOnAxis(ap=emsi[:, i : i + 1], axis=0),
            )
            x3 = x_sb[i].rearrange("p (c h) -> p c h", c=C)
            prod = work.tile([128, C, H], FP32, tag="prod")
            nc.vector.tensor_tensor(
                out=prod, in0=x3, in1=w1g[:, None, :].to_broadcast([128, C, H]),
                op=ALU.mult,
            )
            A = work.tile([128, C], FP32, tag="A")
            nc.vector.reduce_sum(out=A, in_=prod, axis=mybir.AxisListType.X)
            prod2 = work.tile([128, C, H], FP32, tag="prod")
            nc.vector.tensor_tensor(
                out=prod2, in0=x3, in1=w2g[:, None, :].to_broadcast([128, C, H]),
                op=ALU.mult,
            )
            Bt = work.tile([128, C], FP32, tag="B")
            nc.vector.reduce_sum(out=Bt, in_=prod2, axis=mybir.AxisListType.X)
            AB = work.tile([128, C], FP32, tag="AB")
            nc.vector.tensor_tensor(out=AB, in0=A, in1=Bt, op=ALU.mult)
            tsel = work.tile([128, 1], FP32, tag="tsel")
            nc.vector.reduce_sum(out=tsel, in_=AB, axis=mybir.AxisListType.X)
            ntok = work.tile([128, 1], FP32, tag="ntok")
            nc.vector.tensor_tensor(out=ntok, in0=tsel, in1=emg, op=ALU.add)
            nc.vector.tensor_tensor(
                out=ntok, in0=ntok, in1=mf_sb[:, i : i + 1], op=ALU.mult
            )
            nc.tensor.matmul(
                out=psN, lhsT=selc_sb[:, i * 4 : (i + 1) * 4], rhs=ntok,
                start=(i == 0), stop=(i == NCHUNK - 1),
            )

        p1 = tiny.tile([4, 1], FP32, tag="p1")
        nc.vector.tensor_tensor(out=p1, in0=stg, in1=eng, op=ALU.add)
        nc.vector.tensor_tensor(out=p1, in0=p1, in1=em0g, op=ALU.add)
        numt = tiny.tile([4, 1], FP32, tag="numt")
        nc.vector.tensor_tensor(out=numt, in0=psN, in1=p1, op=ALU.add)
        res = tiny.tile([4, 1], FP32, tag="res")
        nc.vector.tensor_tensor(out=res, in0=numt, in1=den4, op=ALU.subtract)
        nc.sync.dma_start(out=loss[:, None], in_=res)

    return nc


def make_core_inputs(inputs, emissions, targets, masks, W1, b1, W2, b2,
                     start_transitions, end_transitions, core):
    b0 = core * BPC
    x = np.ascontiguousarray(inputs[b0 : b0 + BPC]).astype(np.float32)
    em = np.ascontiguousarray(emissions[b0 : b0 + BPC]).astype(np.float32)
    tg = targets[b0 : b0 + BPC].astype(np.int64)
    mk = masks[b0 : b0 + BPC].astype(bool)

    # token col index: ci = b_local*L + l ; chunk i holds ci in [i*128, (i+1)*128)
    bs = np.repeat(np.arange(BPC), L)          # [NTOK]
    ls = np.tile(np.arange(L), BPC)            # [NTOK]
    tgn = tg[bs, ls].astype(np.int32)          # tg[b, l]
    lprev = np.maximum(ls - 1, 0)
    tgp = tg[bs, lprev].astype(np.int32)       # tg[b, l-1] (l=0 dummy)
    mfv = mk[bs, ls].astype(np.float32)
    mfv[ls == 0] = 0.0
    emsel = ((bs * L + ls) * K + tgn).astype(np.int32)

    def colmaj(v, dtype):
        return np.ascontiguousarray(v.reshape(NCHUNK, 128).T.copy()).astype(dtype)

    seq_ends = mk.astype(np.int64).sum(axis=1) - 1
    id128 = np.eye(128, dtype=np.float32)
    selc = np.zeros((128, NCHUNK, 4), dtype=np.float32)
    for i in range(NCHUNK):
        selc[:, i, i // 2] = 1.0
    selc = selc.reshape(128, NCHUNK * 4)
    return {
        "x": x,
        "em": em,
        "W1": W1.astype(np.float32), "W2": W2.astype(np.float32),
        "b1": b1.astype(np.float32), "b2": b2.astype(np.float32),
        "st": start_transitions.astype(np.float32),
        "en": end_transitions.astype(np.float32),
        "id128": id128,
        "selc": selc,
        "mask128": np.kron(np.eye(2), np.ones((64, 1))).astype(np.float32),
        "maskq": np.kron(np.ones((4, 1)), np.kron(np.eye(4), np.ones((8, 1)))).astype(np.float32),
        "maskT2": np.kron(np.eye(2), np.ones((1, 64))).astype(np.float32),
        "mf": colmaj(mfv, np.float32),
        "w1gidx": colmaj(tgp, np.int32),
        "w2gidx": colmaj(tgn, np.int32),
        "emselidx": colmaj(emsel, np.int32),
        "stidx": tg[:, 0].astype(np.int32).reshape(4, 1),
        "enidx": tg[np.arange(BPC), seq_ends].astype(np.int32).reshape(4, 1),
        "em0idx": ((np.arange(BPC) * L + 0) * K + tg[:, 0]).astype(np.int32).reshape(4, 1),
    }


_trace = False


_nc_cache = []


def kernel(**inputs):
    if not _nc_cache:
        nc = bacc.Bacc("TRN2", target_bir_lowering=False, num_devices=NCORES)
        build_program(nc)
        nc.finalize()
        _nc_cache.append(nc)
    nc = _nc_cache[0]
    args = {k: np.asarray(v) for k, v in inputs.items()}
    in_maps = [
        make_core_inputs(
            args["inputs"], args["emissions"], args["targets"], args["masks"],
            args["W1"], args["b1"], args["W2"], args["b2"],
            args["start_transitions"], args["end_transitions"], core,
        )
        for core in range(NCORES)
    ]
    res = run_bass_kernel_spmd(nc, in_maps, core_ids=list(range(NCORES)),
                               trace=_trace)
    out = np.concatenate([res.results[c]["loss"] for c in range(NCORES)])
    if _trace:
        kernel.last_results = res
    return out.astype(np.float32)
